# revision 22
# baseline (speedup 1.0000x reference)
"""Mixtral sparse MoE block on 8 Trainium2 NeuronCores.

Expert-parallel: core e holds expert e's weights (w1/w3/w2 sharded on the E
axis), tokens are dispatched to cores by their top-2 expert assignment
(computed on host from the tiny replicated gate), each core runs the expert
GLU — y = (silu(x w1^T) * (x w3^T)) w2^T — over its token set in fp16 with
fp32 PSUM accumulation (10 mantissa bits; measured 5e-4 rel error), the
weighted combine is a host-side scatter-add.

Device schedule, per core (single pass; the PE matmul stream is the roofline
at ~688k PE cycles = 287us at the 2.4GHz max p-state, everything else hides
behind it; chip-wide DVFS wanders between ~2.0 and 2.4GHz run-to-run, which
moves wall exec 310-373us with identical code):
  Stage 1 keeps tokens in the matmul moving dim (chunks of 512) and produces
  the full actT [F, C] fp16 tensor in SBUF (56 KiB/partition).  Stage 2 flips
  orientation: a 128-token slice of actT is the stationary operand and w2^T
  columns stream at N=512, accumulating all 28 F-tiles into one PSUM bank, so
  the output lands directly in [C, H] layout.  The final tile is split
  384/128 so the post-stream tail is a 128-col cast plus a 32KB write
  (~1us shorter than a full 512-col cast+DMA).

fp16 is the precision floor on TRN2: fp8 DoubleRow matmuls measured exactly
2x fp16 throughput (216ns for K=256 x 512 cols), and every fp8 scheme that
fits the 2e-2 gate needs >=3 matmuls per fp16 matmul (scaled hi/lo residual,
1.7e-3) — strictly slower.  Single-matmul fp8 is 5.4e-2 even with per-tensor
power-of-2 scaling.

DMA queue discipline (HWDGE rings only; SWDGE/gpsimd descriptor generation
for strided transfers measured ~10us issue-to-semaphore and is never used;
both queue preambles end ~7.4us, each DMA config costs ~0.66us of queue
time, completion receipts land 2.8-4.3us after the transfer):
  sync:   f=0 w1 (ko-halves for ring pipelining), f=0 w3, the bulk x
          chunks in ko-halves, HEAD_W-1 prefetch w1/w3 pairs, then the
          pool-paced w1/w3 stream with w2 interleaved in 4-f-tile batches,
          so w2 is fully resident long before stage 2 and never delays
          the w1/w3 prefetch.
  scalar: the first 256-token x chunk alone as ONE piece (a ko-split gate
          that starts the first group on half a chunk was measured to
          stall mid-group and reset the clock ramp whenever the second
          half's receipt slipped), then the fp16 output tiles during
          stage 2 when the ACT engine is idle — bulk DMAs here otherwise
          block the silu stream (same FIFO as ACTIVATE).  Output tiles
          alternate rings so consecutive write receipts overlap.
  x chunks are contiguous-per-partition flat blocks (host pre-layout);
  strided x transfers stall the whole core waiting on their semaphores.
  WARM 256-col warm-up matmuls on a zeroed scratch tile bridge the gap
  between PE-queue-ready (~7-8us) and the first chunk's completion
  semaphore (~11.5-14us), burning the 3us p-state ramp on garbage so the
  real stream starts at full clock and runs gap-free (measured: zero
  >50ns gaps across the ~300us matmul stream on the median run).
"""

import os

if os.environ.get("TRN_TERMINAL_POOL_IPS") and os.environ.get("JAX_PLATFORMS") == "cpu":
    # A cpu-pinned JAX would hide the axon-tunneled NeuronCores this kernel
    # runs on; the devices are reached via jax/PJRT, so let jax see them.
    os.environ.pop("JAX_PLATFORMS")

import numpy as np

import concourse.mybir as mybir
import concourse.tile as tile
from concourse import bacc
from concourse.bass_utils import run_bass_kernel_spmd

H = 1024
F = 3584
E = 8
TOP_K = 2
KO = H // 128     # 8   k-tiles over H (stage-1 contraction)
HB = H // 512     # 2   h-blocks (stage-2 moving dim)
FT = F // 128     # 28  f-tiles over F
S1_CHUNK = 512    # stage-1 moving-dim chunk
HEAD_W = 3        # w1/w3 f-tiles issued ahead of the loop (prefetch depth)
WARM = 24         # 256-col warm-up matmuls bridging preamble -> first data
FILL = 3          # 512-col fillers absorbing the x chunk-1 receipt wait
C_CAP = 1024      # device token capacity; overflow beyond this is tiny and
                  # computed on host

_nc_cache = {}


def _chunks(C, step):
    out = []
    off = 0
    while off < C:
        sz = min(step, C - off)
        out.append((off, sz))
        off += sz
    return out


def _s1_chunks(C):
    # x is laid out chunk-major on the host, so every chunk DMA is one
    # contiguous block per partition (few descriptors): strided x transfers
    # measured ~9us issue-to-semaphore on either DGE path and stalled the
    # whole core.  The first chunk is 256 tokens so its completion semaphore
    # (the gate for the first real matmul group) fires ~1.5us sooner than a
    # 512 chunk's would.
    if C > 512:
        return [(0, 256)] + [(o + 256, s) for o, s in _chunks(C - 256, S1_CHUNK)]
    return _chunks(C, S1_CHUNK)


def _build(C):
    f16, f32 = mybir.dt.float16, mybir.dt.float32
    s1_chunks = _s1_chunks(C)
    TT = C // 128  # token tiles for stage 2

    nc = bacc.Bacc("TRN2", target_bir_lowering=False, debug=False, num_devices=E)
    xb = nc.dram_tensor("xb", [128, KO * C], f16, kind="ExternalInput")
    w1b = nc.dram_tensor("w1b", [FT, 128, KO, 128], f16, kind="ExternalInput")
    w3b = nc.dram_tensor("w3b", [FT, 128, KO, 128], f16, kind="ExternalInput")
    w2b = nc.dram_tensor("w2b", [128, FT, H], f16, kind="ExternalInput")
    yb = nc.dram_tensor("yb", [C, H], f16, kind="ExternalOutput")

    with tile.TileContext(nc) as tc:
        with (
            tc.tile_pool(name="xpool", bufs=1) as xpool,
            tc.tile_pool(name="actpool", bufs=1) as actpool,
            tc.tile_pool(name="w13pool", bufs=4) as w13pool,
            tc.tile_pool(name="w2pool", bufs=1) as w2pool,
            tc.tile_pool(name="outpool", bufs=4) as outpool,
            tc.tile_pool(name="silupool", bufs=4) as silupool,
            tc.tile_pool(name="warmpool", bufs=1) as warmpool,
            tc.tile_pool(name="ps1", bufs=3, space="PSUM") as ps1,
            tc.tile_pool(name="ps2", bufs=2, space="PSUM") as ps2,
        ):
            # Head of the sync queue: the f=0 weights the first matmul group
            # needs, then a HEAD_W-deep prefetch so the PE never starves
            # while the DMA pipeline builds steady state.
            head_w = []
            # First f-tile w1 is split into ko-halves for ring pipelining.
            # The x chunk rides ALONE on the scalar ring so its completion
            # receipt is as fast as possible; all other head traffic goes
            # on sync.  DMA completion receipts degrade when the rings are
            # saturated, so the head burst is kept small (HEAD_W pairs,
            # pool caps the rest).
            w1t0 = w13pool.tile([128, KO, 128], f16, tag="w1t", name="w1t0")
            nc.sync.dma_start(w1t0[:, : KO // 2], w1b[0][:, : KO // 2])
            xt = xpool.tile([128, KO * C], f16)
            co, cs = s1_chunks[0]
            # Chunk 0 rides ALONE on the scalar ring as ONE piece: receipts
            # land ~2.8us after the transfer regardless, and a ko-split gate
            # that lets the first group start on half a chunk was measured
            # to stall mid-group (and reset the clock ramp) whenever the
            # second half's receipt slipped.
            nc.scalar.dma_start(
                xt[:, KO * co : KO * (co + cs)], xb[:, KO * co : KO * (co + cs)]
            )
            nc.sync.dma_start(w1t0[:, KO // 2 :], w1b[0][:, KO // 2 :])
            w3t0 = w13pool.tile([128, KO, 128], f16, tag="w3t", name="w3t0")
            nc.sync.dma_start(w3t0[:], w3b[0])
            head_w.append((w1t0, w3t0))
            # Prefetch pairs BEFORE the bulk x chunks: with the chunk-0-
            # first stage-1 order, f1/f2 weights are needed at stream+1.7us
            # and +3.4us while the x chunks 1/2 are not needed until
            # stream+5.1us and +8.5us — so the weight pairs must beat the
            # 1.5MB of bulk x onto the sync ring.  Each bulk chunk is
            # ko-split so its first matmul group gates on half the bytes.
            for f in range(1, HEAD_W):
                w1tf = w13pool.tile([128, KO, 128], f16, tag="w1t", name="w1th")
                nc.sync.dma_start(w1tf[:], w1b[f])
                w3tf = w13pool.tile([128, KO, 128], f16, tag="w3t", name="w3th")
                nc.sync.dma_start(w3tf[:], w3b[f])
                head_w.append((w1tf, w3tf))
            # Chunk 1 rides the scalar ring behind chunk 0 (the ring is
            # otherwise idle until the stage-2 output tiles ~200us later),
            # so its receipt beats the stream+5.1us deadline instead of
            # queueing behind ~3MB of weight traffic on sync.  Chunk 2+
            # stays on sync (deadline stream+8.5us, met comfortably).
            for ci, (co, cs) in enumerate(s1_chunks[1:], start=1):
                q = nc.scalar if ci == 1 else nc.sync
                half = KO // 2 * cs
                q.dma_start(
                    xt[:, KO * co : KO * co + half], xb[:, KO * co : KO * co + half]
                )
                q.dma_start(
                    xt[:, KO * co + half : KO * (co + cs)],
                    xb[:, KO * co + half : KO * (co + cs)],
                )

            # Warm-up matmuls bridge the ~4us between the engine-sync
            # barrier and the first weight DMA's completion semaphore, so
            # the PE HAM activity window opens as early as possible and the
            # real stream starts the moment data is visible.
            warm = warmpool.tile([128, 512], mybir.dt.bfloat16)
            nc.vector.memset(warm[:], 0.0)
            # WARM 256-col warm-up matmuls bridge preamble-end to the
            # arrival of the first x/weight completion semaphores — fewer
            # leaves the PE idle at the handoff, more overshoots past
            # data-ready.  256-col granularity keeps the overshoot small.
            for _ in range(WARM):
                wp = ps2.tile([128, 512], f32, tag="py", name="wp")
                nc.tensor.matmul(
                    wp[:, :256], warm[:, :128], warm[:, :256], start=True, stop=True
                )

            act = actpool.tile([128, FT, C], f16, tag="act")
            w2t = w2pool.tile([128, FT, H], f16, tag="w2t")

            # Stage 1: actT[f, c] = silu(w1 xT) * (w3 xT), per 128-row f tile.
            # Iteration order: chunk 0 across the HEAD_W resident f-tiles
            # FIRST (gated only on the head DMAs), then their chunks 1/2,
            # then f >= HEAD_W in normal order.  This pushes the x chunk-1
            # receipt deadline from stream+1.7us to stream+5.1us, so no
            # filler matmuls are needed and a late receipt cannot gap the
            # PE.  Within the catch-up block f-major order (f0c1, f0c2,
            # f1c1, ...) retires f0's weight tile early so f3's pool
            # buffer recycle (WAR on f0's last read) cannot delay its DMA.
            nch = len(s1_chunks)
            if nch > 1:
                s1_order = [(f, 0) for f in range(HEAD_W)]
                s1_order += [
                    (f, ci) for f in range(HEAD_W) for ci in range(1, nch)
                ]
                s1_order += [
                    (f, ci) for f in range(HEAD_W, FT) for ci in range(nch)
                ]
            else:
                s1_order = [(f, 0) for f in range(FT)]
            cur_w = {}
            for f, ci in s1_order:
                co, cs = s1_chunks[ci]
                if f < HEAD_W:
                    w1t, w3t = head_w[f]
                elif f in cur_w:
                    w1t, w3t = cur_w[f]
                else:
                    w1t = w13pool.tile([128, KO, 128], f16, tag="w1t", name="w1t")
                    nc.sync.dma_start(w1t[:], w1b[f])
                    w3t = w13pool.tile([128, KO, 128], f16, tag="w3t", name="w3t")
                    nc.sync.dma_start(w3t[:], w3b[f])
                    cur_w = {f: (w1t, w3t)}
                    # Interleave w2 in 4-f-tile batches behind the weight
                    # stream; all of w2 is resident well before stage 2.
                    fw = f - HEAD_W
                    if fw % 4 == 0 and fw + 4 <= FT:
                        nc.sync.dma_start(w2t[:, fw : fw + 4], w2b[:, fw : fw + 4])
                if True:
                    p1 = ps1.tile([128, S1_CHUNK], f32, tag="p1", name="p1")[:, :cs]
                    p3 = ps1.tile([128, S1_CHUNK], f32, tag="p3", name="p3")[:, :cs]
                    for ko in range(KO):
                        xs = xt[:, KO * co + ko * cs : KO * co + (ko + 1) * cs]
                        nc.tensor.matmul(
                            p1, w1t[:, ko], xs,
                            start=(ko == 0), stop=(ko == KO - 1),
                        )
                    for ko in range(KO):
                        xs = xt[:, KO * co + ko * cs : KO * co + (ko + 1) * cs]
                        nc.tensor.matmul(
                            p3, w3t[:, ko], xs,
                            start=(ko == 0), stop=(ko == KO - 1),
                        )
                    st = silupool.tile([128, S1_CHUNK], f32, tag="st", name="st")[:, :cs]
                    nc.scalar.activation(
                        st, p1, mybir.ActivationFunctionType.Silu
                    )
                    nc.vector.tensor_tensor(
                        act[:, f, co : co + cs], st, p3, mybir.AluOpType.mult
                    )
            # (w2 fully covered by the 4-f-tile batches above: fw = 0,4,..,24)

            # Stage 2: y[tok, h] += actT[:, tok-tile].T @ w2T[:, h-block],
            # accumulating all 28 f-tiles in one PSUM bank.
            for t in range(TT):
                ts = slice(t * 128, (t + 1) * 128)
                for hb in range(HB):
                    hs = slice(hb * 512, (hb + 1) * 512)
                    last = t == TT - 1 and hb == HB - 1
                    if not last:
                        py = ps2.tile([128, 512], f32, tag="py", name="py")
                        for kf in range(FT):
                            nc.tensor.matmul(
                                py, act[:, kf, ts], w2t[:, kf, hs],
                                start=(kf == 0), stop=(kf == FT - 1),
                            )
                        osb = outpool.tile([128, 512], f16, tag="osb", name="osb")
                        nc.vector.tensor_copy(osb[:], py[:])
                        # Alternate output rings so consecutive tiles' write
                        # receipts overlap.
                        q = nc.sync if hb == 0 else nc.scalar
                        q.dma_start(yb[ts, hs], osb[:])
                    else:
                        # Final tile 384/128-split: the 384-col cast+DMA
                        # hide behind the 128-col group's matmuls, so the
                        # post-stream tail is only a 128-col cast plus a
                        # 32KB write instead of a full 512-col tile.
                        h0 = hb * 512
                        pya = ps2.tile([128, 512], f32, tag="py", name="pya")
                        for kf in range(FT):
                            nc.tensor.matmul(
                                pya[:, :384], act[:, kf, ts], w2t[:, kf, h0 : h0 + 384],
                                start=(kf == 0), stop=(kf == FT - 1),
                            )
                        osba = outpool.tile([128, 512], f16, tag="osb", name="osba")
                        nc.vector.tensor_copy(osba[:, :384], pya[:, :384])
                        nc.scalar.dma_start(yb[ts, h0 : h0 + 384], osba[:, :384])
                        pyb = ps2.tile([128, 512], f32, tag="py", name="pyb")
                        for kf in range(FT):
                            nc.tensor.matmul(
                                pyb[:, :128], act[:, kf, ts],
                                w2t[:, kf, h0 + 384 : h0 + 512],
                                start=(kf == 0), stop=(kf == FT - 1),
                            )
                        osbb = outpool.tile([128, 512], f16, tag="osb", name="osbb")
                        nc.vector.tensor_copy(osbb[:, :128], pyb[:, :128])
                        nc.sync.dma_start(yb[ts, h0 + 384 : h0 + 512], osbb[:, :128])
    nc.compile()
    return nc


def _routing(x, gate_w):
    """Replicates the reference router in fp32 numpy: softmax over expert
    logits, top-2, renormalized weights.  Verified to match jax bit-for-bit
    on expert selection for these inputs (min top2/top3 prob gap 3e-5)."""
    logits = x @ gate_w.T
    m = logits.max(-1, keepdims=True)
    p = np.exp(logits - m)
    p /= p.sum(-1, keepdims=True)
    top_i = np.argsort(-p, axis=-1, kind="stable")[:, :TOP_K]
    top_v = np.take_along_axis(p, top_i, axis=-1)
    top_v = top_v / top_v.sum(-1, keepdims=True)
    return top_i, top_v


def kernel(hidden_states, gate_w, w1, w3, w2):
    B, S, _ = hidden_states.shape
    x = np.ascontiguousarray(
        np.asarray(hidden_states, dtype=np.float32).reshape(-1, H)
    )
    gate_w = np.asarray(gate_w, dtype=np.float32)
    w1 = np.asarray(w1, dtype=np.float32)
    w3 = np.asarray(w3, dtype=np.float32)
    w2 = np.asarray(w2, dtype=np.float32)
    T = x.shape[0]

    top_i, top_v = _routing(x, gate_w)

    idx = [np.flatnonzero((top_i == e).any(axis=1)) for e in range(E)]
    wgt = []
    for e in range(E):
        sel = top_i[idx[e]] == e
        wgt.append(
            np.take_along_axis(top_v[idx[e]], np.argmax(sel, 1)[:, None], 1)[:, 0]
        )

    cmax = max(len(i) for i in idx)
    C = min(max(((cmax + 127) // 128) * 128, 128), C_CAP)
    n_dev = [min(len(i), C) for i in idx]

    if C not in _nc_cache:
        _nc_cache[C] = _build(C)
    nc = _nc_cache[C]

    in_maps = []
    for e in range(E):
        x_pad = np.zeros((C, H), dtype=np.float32)
        x_pad[: n_dev[e]] = x[idx[e][: n_dev[e]]]
        # Chunk-major flat layout [128 partition, KO*C]: each 512-token
        # chunk is one contiguous block per partition, so its DMA needs few
        # descriptors and completes (semaphore included) promptly.
        xb = np.concatenate(
            [
                x_pad[co : co + cs]
                .T.reshape(KO, 128, cs)
                .transpose(1, 0, 2)
                .reshape(128, KO * cs)
                for co, cs in _s1_chunks(C)
            ],
            axis=1,
        )
        t1 = w1[e].reshape(FT, 128, KO, 128)
        w1b = np.ascontiguousarray(t1.transpose(0, 3, 2, 1))
        t3 = w3[e].reshape(FT, 128, KO, 128)
        w3b = np.ascontiguousarray(t3.transpose(0, 3, 2, 1))
        w2b = np.ascontiguousarray(w2[e].T.reshape(FT, 128, H).transpose(1, 0, 2))
        in_maps.append({"xb": xb.astype(np.float16), "w1b": w1b.astype(np.float16), "w3b": w3b.astype(np.float16), "w2b": w2b.astype(np.float16)})

    res = run_bass_kernel_spmd(nc, in_maps, core_ids=list(range(E)))

    out = np.zeros((T, H), dtype=np.float32)
    for e in range(E):
        y_e = res.results[e]["yb"].astype(np.float32)  # [C, H] fp16 on device
        out[idx[e][: n_dev[e]]] += wgt[e][: n_dev[e], None] * y_e[: n_dev[e]]
        if len(idx[e]) > n_dev[e]:
            # Overflow tokens past the capacity grid (a percent or so in the
            # worst-loaded expert): exact fp32 on host.
            xo = x[idx[e][n_dev[e] :]]
            h1 = xo @ w1[e].T
            a = (h1 / (1.0 + np.exp(-h1))) * (xo @ w3[e].T)
            yo = a @ w2[e].T
            out[idx[e][n_dev[e] :]] += wgt[e][n_dev[e] :, None] * yo
    return out.reshape(B, S, H)



# revision 23
# speedup vs baseline: 1.0055x; 1.0055x over previous
"""Mixtral sparse MoE block on 8 Trainium2 NeuronCores.

Expert-parallel: core e holds expert e's weights (w1/w3/w2 sharded on the E
axis), tokens are dispatched to cores by their top-2 expert assignment
(computed on host from the tiny replicated gate), each core runs the expert
GLU — y = (silu(x w1^T) * (x w3^T)) w2^T — over its token set in fp16 with
fp32 PSUM accumulation (10 mantissa bits; measured 5e-4 rel error), the
weighted combine is a host-side scatter-add.

Device schedule, per core (single pass; the PE matmul stream is the roofline
at ~688k PE cycles = 287us at the 2.4GHz max p-state, everything else hides
behind it; chip-wide DVFS wanders between ~2.0 and 2.4GHz run-to-run, which
moves wall exec 310-373us with identical code):
  Stage 1 keeps tokens in the matmul moving dim (chunks of 512) and produces
  the full actT [F, C] fp16 tensor in SBUF (56 KiB/partition).  Stage 2 flips
  orientation: a 128-token slice of actT is the stationary operand and w2^T
  columns stream at N=512, accumulating all 28 F-tiles into one PSUM bank, so
  the output lands directly in [C, H] layout.  The final tile is split
  384/128 so the post-stream tail is a 128-col cast plus a 32KB write
  (~1us shorter than a full 512-col cast+DMA).

fp16 is the precision floor on TRN2: fp8 DoubleRow matmuls measured exactly
2x fp16 throughput (216ns for K=256 x 512 cols), and every fp8 scheme that
fits the 2e-2 gate needs >=3 matmuls per fp16 matmul (scaled hi/lo residual,
1.7e-3) — strictly slower.  Single-matmul fp8 is 5.4e-2 even with per-tensor
power-of-2 scaling.

DMA queue discipline (HWDGE rings only; SWDGE/gpsimd descriptor generation
for strided transfers measured ~10us issue-to-semaphore and is never used;
both queue preambles end ~7.4us, each DMA config costs ~0.66us of queue
time, completion receipts land 2.8-4.3us after the transfer):
  sync:   f=0 w1 (ko-halves for ring pipelining), f=0 w3, the bulk x
          chunks in ko-halves, HEAD_W-1 prefetch w1/w3 pairs, then the
          pool-paced w1/w3 stream with w2 interleaved in 4-f-tile batches,
          so w2 is fully resident long before stage 2 and never delays
          the w1/w3 prefetch.
  scalar: the first 256-token x chunk alone as ONE piece (a ko-split gate
          that starts the first group on half a chunk was measured to
          stall mid-group and reset the clock ramp whenever the second
          half's receipt slipped), then the fp16 output tiles during
          stage 2 when the ACT engine is idle — bulk DMAs here otherwise
          block the silu stream (same FIFO as ACTIVATE).  Output tiles
          alternate rings so consecutive write receipts overlap.
  x chunks are contiguous-per-partition flat blocks (host pre-layout);
  strided x transfers stall the whole core waiting on their semaphores.
  WARM 256-col warm-up matmuls on a zeroed scratch tile bridge the gap
  between PE-queue-ready (~7-8us) and the first chunk's completion
  semaphore (~11.5-14us), burning the 3us p-state ramp on garbage so the
  real stream starts at full clock and runs gap-free (measured: zero
  >50ns gaps across the ~300us matmul stream on the median run).
"""

import os

if os.environ.get("TRN_TERMINAL_POOL_IPS") and os.environ.get("JAX_PLATFORMS") == "cpu":
    # A cpu-pinned JAX would hide the axon-tunneled NeuronCores this kernel
    # runs on; the devices are reached via jax/PJRT, so let jax see them.
    os.environ.pop("JAX_PLATFORMS")

import numpy as np

import concourse.mybir as mybir
import concourse.tile as tile
from concourse import bacc
from concourse.bass_utils import run_bass_kernel_spmd

H = 1024
F = 3584
E = 8
TOP_K = 2
KO = H // 128     # 8   k-tiles over H (stage-1 contraction)
HB = H // 512     # 2   h-blocks (stage-2 moving dim)
FT = F // 128     # 28  f-tiles over F
S1_CHUNK = 512    # stage-1 moving-dim chunk
HEAD_W = 3        # w1/w3 f-tiles issued ahead of the loop (prefetch depth)
WARM = 24         # 256-col warm-up matmuls bridging preamble -> first data
C_CAP = 1024      # device token capacity; overflow beyond this is tiny and
                  # computed on host

_nc_cache = {}


def _chunks(C, step):
    out = []
    off = 0
    while off < C:
        sz = min(step, C - off)
        out.append((off, sz))
        off += sz
    return out


def _s1_chunks(C):
    # x is laid out chunk-major on the host, so every chunk DMA is one
    # contiguous block per partition (few descriptors): strided x transfers
    # measured ~9us issue-to-semaphore on either DGE path and stalled the
    # whole core.  The first chunk is 256 tokens so its completion semaphore
    # (the gate for the first real matmul group) fires ~1.5us sooner than a
    # 512 chunk's would.
    if C > 512:
        return [(0, 256)] + [(o + 256, s) for o, s in _chunks(C - 256, S1_CHUNK)]
    return _chunks(C, S1_CHUNK)


def _build(C):
    f16, f32 = mybir.dt.float16, mybir.dt.float32
    s1_chunks = _s1_chunks(C)
    TT = C // 128  # token tiles for stage 2

    nc = bacc.Bacc("TRN2", target_bir_lowering=False, debug=False, num_devices=E)
    xb = nc.dram_tensor("xb", [128, KO * C], f16, kind="ExternalInput")
    w1b = nc.dram_tensor("w1b", [FT, 128, KO, 128], f16, kind="ExternalInput")
    w3b = nc.dram_tensor("w3b", [FT, 128, KO, 128], f16, kind="ExternalInput")
    w2b = nc.dram_tensor("w2b", [128, FT, H], f16, kind="ExternalInput")
    yb = nc.dram_tensor("yb", [C, H], f16, kind="ExternalOutput")

    with tile.TileContext(nc) as tc:
        with (
            tc.tile_pool(name="xpool", bufs=1) as xpool,
            tc.tile_pool(name="actpool", bufs=1) as actpool,
            tc.tile_pool(name="w13pool", bufs=4) as w13pool,
            tc.tile_pool(name="w2pool", bufs=1) as w2pool,
            tc.tile_pool(name="outpool", bufs=4) as outpool,
            tc.tile_pool(name="silupool", bufs=4) as silupool,
            tc.tile_pool(name="warmpool", bufs=1) as warmpool,
            tc.tile_pool(name="ps1", bufs=3, space="PSUM") as ps1,
            tc.tile_pool(name="ps2", bufs=2, space="PSUM") as ps2,
        ):
            # Head of the sync queue: the f=0 weights the first matmul group
            # needs, then a HEAD_W-deep prefetch so the PE never starves
            # while the DMA pipeline builds steady state.
            head_w = []
            # First f-tile w1 is split into ko-halves for ring pipelining.
            # The x chunk rides ALONE on the scalar ring so its completion
            # receipt is as fast as possible; all other head traffic goes
            # on sync.  DMA completion receipts degrade when the rings are
            # saturated, so the head burst is kept small (HEAD_W pairs,
            # pool caps the rest).
            w1t0 = w13pool.tile([128, KO, 128], f16, tag="w1t", name="w1t0")
            nc.sync.dma_start(w1t0[:, : KO // 2], w1b[0][:, : KO // 2])
            xt = xpool.tile([128, KO * C], f16)
            co, cs = s1_chunks[0]
            # Chunk 0 rides ALONE on the scalar ring as ONE piece: receipts
            # land ~2.8us after the transfer regardless, and a ko-split gate
            # that lets the first group start on half a chunk was measured
            # to stall mid-group (and reset the clock ramp) whenever the
            # second half's receipt slipped.
            nc.scalar.dma_start(
                xt[:, KO * co : KO * (co + cs)], xb[:, KO * co : KO * (co + cs)]
            )
            nc.sync.dma_start(w1t0[:, KO // 2 :], w1b[0][:, KO // 2 :])
            w3t0 = w13pool.tile([128, KO, 128], f16, tag="w3t", name="w3t0")
            nc.sync.dma_start(w3t0[:], w3b[0])
            head_w.append((w1t0, w3t0))
            # Prefetch pairs BEFORE the bulk x chunks: with the chunk-0-
            # first stage-1 order, f1/f2 weights are needed at stream+1.7us
            # and +3.4us while the x chunks 1/2 are not needed until
            # stream+5.1us and +8.5us — so the weight pairs must beat the
            # 1.5MB of bulk x onto the sync ring.  Each bulk chunk is
            # ko-split so its first matmul group gates on half the bytes.
            for f in range(1, HEAD_W):
                w1tf = w13pool.tile([128, KO, 128], f16, tag="w1t", name="w1th")
                nc.sync.dma_start(w1tf[:], w1b[f])
                w3tf = w13pool.tile([128, KO, 128], f16, tag="w3t", name="w3th")
                nc.sync.dma_start(w3tf[:], w3b[f])
                head_w.append((w1tf, w3tf))
            # Chunk 1 rides the scalar ring behind chunk 0 (the ring is
            # otherwise idle until the stage-2 output tiles ~200us later),
            # so its receipt beats the stream+5.1us deadline instead of
            # queueing behind ~3MB of weight traffic on sync.  Chunk 2+
            # stays on sync (deadline stream+8.5us, met comfortably).
            for ci, (co, cs) in enumerate(s1_chunks[1:], start=1):
                q = nc.scalar if ci == 1 else nc.sync
                half = KO // 2 * cs
                q.dma_start(
                    xt[:, KO * co : KO * co + half], xb[:, KO * co : KO * co + half]
                )
                q.dma_start(
                    xt[:, KO * co + half : KO * (co + cs)],
                    xb[:, KO * co + half : KO * (co + cs)],
                )

            # Warm-up matmuls bridge the ~4us between the engine-sync
            # barrier and the first weight DMA's completion semaphore, so
            # the PE HAM activity window opens as early as possible and the
            # real stream starts the moment data is visible.
            warm = warmpool.tile([128, 512], mybir.dt.bfloat16)
            nc.vector.memset(warm[:], 0.0)
            # WARM 256-col warm-up matmuls bridge preamble-end to the
            # arrival of the first x/weight completion semaphores — fewer
            # leaves the PE idle at the handoff, more overshoots past
            # data-ready.  256-col granularity keeps the overshoot small.
            for _ in range(WARM):
                wp = ps2.tile([128, 512], f32, tag="py", name="wp")
                nc.tensor.matmul(
                    wp[:, :256], warm[:, :128], warm[:, :256], start=True, stop=True
                )

            act = actpool.tile([128, FT, C], f16, tag="act")
            w2t = w2pool.tile([128, FT, H], f16, tag="w2t")

            # Stage 1: actT[f, c] = silu(w1 xT) * (w3 xT), per 128-row f tile.
            # Iteration order: chunk 0 across the HEAD_W resident f-tiles
            # FIRST (gated only on the head DMAs), then their chunks 1/2,
            # then f >= HEAD_W in normal order.  This pushes the x chunk-1
            # receipt deadline from stream+1.7us to stream+5.1us, so no
            # filler matmuls are needed and a late receipt cannot gap the
            # PE.  Within the catch-up block f-major order (f0c1, f0c2,
            # f1c1, ...) retires f0's weight tile early so f3's pool
            # buffer recycle (WAR on f0's last read) cannot delay its DMA.
            nch = len(s1_chunks)
            if nch > 1:
                s1_order = [(f, 0) for f in range(HEAD_W)]
                s1_order += [
                    (f, ci) for f in range(HEAD_W) for ci in range(1, nch)
                ]
                s1_order += [
                    (f, ci) for f in range(HEAD_W, FT) for ci in range(nch)
                ]
            else:
                s1_order = [(f, 0) for f in range(FT)]
            cur_w = {}
            for f, ci in s1_order:
                co, cs = s1_chunks[ci]
                if f < HEAD_W:
                    w1t, w3t = head_w[f]
                elif f in cur_w:
                    w1t, w3t = cur_w[f]
                else:
                    w1t = w13pool.tile([128, KO, 128], f16, tag="w1t", name="w1t")
                    nc.sync.dma_start(w1t[:], w1b[f])
                    w3t = w13pool.tile([128, KO, 128], f16, tag="w3t", name="w3t")
                    nc.sync.dma_start(w3t[:], w3b[f])
                    cur_w = {f: (w1t, w3t)}
                    # Interleave w2 in 4-f-tile batches behind the weight
                    # stream; all of w2 is resident well before stage 2.
                    fw = f - HEAD_W
                    if fw % 4 == 0 and fw + 4 <= FT:
                        nc.sync.dma_start(w2t[:, fw : fw + 4], w2b[:, fw : fw + 4])
                if True:
                    p1 = ps1.tile([128, S1_CHUNK], f32, tag="p1", name="p1")[:, :cs]
                    p3 = ps1.tile([128, S1_CHUNK], f32, tag="p3", name="p3")[:, :cs]
                    for ko in range(KO):
                        xs = xt[:, KO * co + ko * cs : KO * co + (ko + 1) * cs]
                        nc.tensor.matmul(
                            p1, w1t[:, ko], xs,
                            start=(ko == 0), stop=(ko == KO - 1),
                        )
                    for ko in range(KO):
                        xs = xt[:, KO * co + ko * cs : KO * co + (ko + 1) * cs]
                        nc.tensor.matmul(
                            p3, w3t[:, ko], xs,
                            start=(ko == 0), stop=(ko == KO - 1),
                        )
                    st = silupool.tile([128, S1_CHUNK], f32, tag="st", name="st")[:, :cs]
                    nc.scalar.activation(
                        st, p1, mybir.ActivationFunctionType.Silu
                    )
                    nc.vector.tensor_tensor(
                        act[:, f, co : co + cs], st, p3, mybir.AluOpType.mult
                    )
            # (w2 fully covered by the 4-f-tile batches above: fw = 0,4,..,24)

            # Stage 2: y[tok, h] += actT[:, tok-tile].T @ w2T[:, h-block],
            # accumulating all 28 f-tiles in one PSUM bank.
            for t in range(TT):
                ts = slice(t * 128, (t + 1) * 128)
                for hb in range(HB):
                    hs = slice(hb * 512, (hb + 1) * 512)
                    last = t == TT - 1 and hb == HB - 1
                    if not last:
                        py = ps2.tile([128, 512], f32, tag="py", name="py")
                        for kf in range(FT):
                            nc.tensor.matmul(
                                py, act[:, kf, ts], w2t[:, kf, hs],
                                start=(kf == 0), stop=(kf == FT - 1),
                            )
                        osb = outpool.tile([128, 512], f16, tag="osb", name="osb")
                        nc.vector.tensor_copy(osb[:], py[:])
                        # Alternate output rings so consecutive tiles' write
                        # receipts overlap.
                        q = nc.sync if hb == 0 else nc.scalar
                        q.dma_start(yb[ts, hs], osb[:])
                    else:
                        # Final tile 384/128-split: the 384-col cast+DMA
                        # hide behind the 128-col group's matmuls, so the
                        # post-stream tail is only a 128-col cast plus a
                        # 32KB write instead of a full 512-col tile.
                        h0 = hb * 512
                        pya = ps2.tile([128, 512], f32, tag="py", name="pya")
                        for kf in range(FT):
                            nc.tensor.matmul(
                                pya[:, :384], act[:, kf, ts], w2t[:, kf, h0 : h0 + 384],
                                start=(kf == 0), stop=(kf == FT - 1),
                            )
                        osba = outpool.tile([128, 512], f16, tag="osb", name="osba")
                        nc.vector.tensor_copy(osba[:, :384], pya[:, :384])
                        nc.scalar.dma_start(yb[ts, h0 : h0 + 384], osba[:, :384])
                        pyb = ps2.tile([128, 512], f32, tag="py", name="pyb")
                        for kf in range(FT):
                            nc.tensor.matmul(
                                pyb[:, :128], act[:, kf, ts],
                                w2t[:, kf, h0 + 384 : h0 + 512],
                                start=(kf == 0), stop=(kf == FT - 1),
                            )
                        osbb = outpool.tile([128, 512], f16, tag="osb", name="osbb")
                        nc.vector.tensor_copy(osbb[:, :128], pyb[:, :128])
                        nc.sync.dma_start(yb[ts, h0 + 384 : h0 + 512], osbb[:, :128])
    nc.compile()
    return nc


def _routing(x, gate_w):
    """Replicates the reference router in fp32 numpy: softmax over expert
    logits, top-2, renormalized weights.  Verified to match jax bit-for-bit
    on expert selection for these inputs (min top2/top3 prob gap 3e-5)."""
    logits = x @ gate_w.T
    m = logits.max(-1, keepdims=True)
    p = np.exp(logits - m)
    p /= p.sum(-1, keepdims=True)
    top_i = np.argsort(-p, axis=-1, kind="stable")[:, :TOP_K]
    top_v = np.take_along_axis(p, top_i, axis=-1)
    top_v = top_v / top_v.sum(-1, keepdims=True)
    return top_i, top_v


def kernel(hidden_states, gate_w, w1, w3, w2):
    B, S, _ = hidden_states.shape
    x = np.ascontiguousarray(
        np.asarray(hidden_states, dtype=np.float32).reshape(-1, H)
    )
    gate_w = np.asarray(gate_w, dtype=np.float32)
    w1 = np.asarray(w1, dtype=np.float32)
    w3 = np.asarray(w3, dtype=np.float32)
    w2 = np.asarray(w2, dtype=np.float32)
    T = x.shape[0]

    top_i, top_v = _routing(x, gate_w)

    idx = [np.flatnonzero((top_i == e).any(axis=1)) for e in range(E)]
    wgt = []
    for e in range(E):
        sel = top_i[idx[e]] == e
        wgt.append(
            np.take_along_axis(top_v[idx[e]], np.argmax(sel, 1)[:, None], 1)[:, 0]
        )

    cmax = max(len(i) for i in idx)
    C = min(max(((cmax + 127) // 128) * 128, 128), C_CAP)
    n_dev = [min(len(i), C) for i in idx]

    if C not in _nc_cache:
        _nc_cache[C] = _build(C)
    nc = _nc_cache[C]

    in_maps = []
    for e in range(E):
        x_pad = np.zeros((C, H), dtype=np.float32)
        x_pad[: n_dev[e]] = x[idx[e][: n_dev[e]]]
        # Chunk-major flat layout [128 partition, KO*C]: each 512-token
        # chunk is one contiguous block per partition, so its DMA needs few
        # descriptors and completes (semaphore included) promptly.
        xb = np.concatenate(
            [
                x_pad[co : co + cs]
                .T.reshape(KO, 128, cs)
                .transpose(1, 0, 2)
                .reshape(128, KO * cs)
                for co, cs in _s1_chunks(C)
            ],
            axis=1,
        )
        t1 = w1[e].reshape(FT, 128, KO, 128)
        w1b = np.ascontiguousarray(t1.transpose(0, 3, 2, 1))
        t3 = w3[e].reshape(FT, 128, KO, 128)
        w3b = np.ascontiguousarray(t3.transpose(0, 3, 2, 1))
        w2b = np.ascontiguousarray(w2[e].T.reshape(FT, 128, H).transpose(1, 0, 2))
        in_maps.append({"xb": xb.astype(np.float16), "w1b": w1b.astype(np.float16), "w3b": w3b.astype(np.float16), "w2b": w2b.astype(np.float16)})

    res = run_bass_kernel_spmd(nc, in_maps, core_ids=list(range(E)))

    out = np.zeros((T, H), dtype=np.float32)
    for e in range(E):
        y_e = res.results[e]["yb"].astype(np.float32)  # [C, H] fp16 on device
        out[idx[e][: n_dev[e]]] += wgt[e][: n_dev[e], None] * y_e[: n_dev[e]]
        if len(idx[e]) > n_dev[e]:
            # Overflow tokens past the capacity grid (a percent or so in the
            # worst-loaded expert): exact fp32 on host.
            xo = x[idx[e][n_dev[e] :]]
            h1 = xo @ w1[e].T
            a = (h1 / (1.0 + np.exp(-h1))) * (xo @ w3[e].T)
            yo = a @ w2[e].T
            out[idx[e][n_dev[e] :]] += wgt[e][n_dev[e] :, None] * yo
    return out.reshape(B, S, H)



# revision 24
# speedup vs baseline: 1.0066x; 1.0011x over previous
"""Mixtral sparse MoE block on 8 Trainium2 NeuronCores.

Expert-parallel: core e holds expert e's weights (w1/w3/w2 sharded on the E
axis), tokens are dispatched to cores by their top-2 expert assignment
(computed on host from the tiny replicated gate), each core runs the expert
GLU — y = (silu(x w1^T) * (x w3^T)) w2^T — over its token set in fp16 with
fp32 PSUM accumulation (10 mantissa bits; measured 5e-4 rel error), the
weighted combine is a host-side scatter-add.

Device schedule, per core (single pass; the PE matmul stream is the roofline
at ~688k PE cycles = 287us at the 2.4GHz max p-state, everything else hides
behind it; chip-wide DVFS wanders between ~2.0 and 2.4GHz run-to-run, which
moves wall exec 310-373us with identical code):
  Stage 1 keeps tokens in the matmul moving dim (chunks of 512) and produces
  the full actT [F, C] fp16 tensor in SBUF (56 KiB/partition).  Stage 2 flips
  orientation: a 128-token slice of actT is the stationary operand and w2^T
  columns stream at N=512, accumulating all 28 F-tiles into one PSUM bank, so
  the output lands directly in [C, H] layout.  The final tile is split
  384/128 so the post-stream tail is a 128-col cast plus a 32KB write
  (~1us shorter than a full 512-col cast+DMA).

fp16 is the precision floor on TRN2: fp8 DoubleRow matmuls measured exactly
2x fp16 throughput (216ns for K=256 x 512 cols), and every fp8 scheme that
fits the 2e-2 gate needs >=3 matmuls per fp16 matmul (scaled hi/lo residual,
1.7e-3) — strictly slower.  Single-matmul fp8 is 5.4e-2 even with per-tensor
power-of-2 scaling.

DMA queue discipline (HWDGE rings only; SWDGE/gpsimd descriptor generation
for strided transfers measured ~10us issue-to-semaphore and is never used;
both queue preambles end ~7.4us, each DMA config costs ~0.66us of queue
time, completion receipts land 2.8-4.3us after the transfer):
  sync:   f=0 w1 (ko-halves for ring pipelining), f=0 w3, the bulk x
          chunks in ko-halves, HEAD_W-1 prefetch w1/w3 pairs, then the
          pool-paced w1/w3 stream with w2 interleaved in 4-f-tile batches,
          so w2 is fully resident long before stage 2 and never delays
          the w1/w3 prefetch.
  scalar: the first 256-token x chunk alone as ONE piece (a ko-split gate
          that starts the first group on half a chunk was measured to
          stall mid-group and reset the clock ramp whenever the second
          half's receipt slipped), then the fp16 output tiles during
          stage 2 when the ACT engine is idle — bulk DMAs here otherwise
          block the silu stream (same FIFO as ACTIVATE).  Output tiles
          alternate rings so consecutive write receipts overlap.
  x chunks are contiguous-per-partition flat blocks (host pre-layout);
  strided x transfers stall the whole core waiting on their semaphores.
  WARM 256-col warm-up matmuls on a zeroed scratch tile bridge the gap
  between PE-queue-ready (~7-8us) and the first chunk's completion
  semaphore (~11.5-14us), burning the 3us p-state ramp on garbage so the
  real stream starts at full clock and runs gap-free (measured: zero
  >50ns gaps across the ~300us matmul stream on the median run).
"""

import os

if os.environ.get("TRN_TERMINAL_POOL_IPS") and os.environ.get("JAX_PLATFORMS") == "cpu":
    # A cpu-pinned JAX would hide the axon-tunneled NeuronCores this kernel
    # runs on; the devices are reached via jax/PJRT, so let jax see them.
    os.environ.pop("JAX_PLATFORMS")

import numpy as np

import concourse.mybir as mybir
import concourse.tile as tile
from concourse import bacc
from concourse.bass_utils import run_bass_kernel_spmd

H = 1024
F = 3584
E = 8
TOP_K = 2
KO = H // 128     # 8   k-tiles over H (stage-1 contraction)
HB = H // 512     # 2   h-blocks (stage-2 moving dim)
FT = F // 128     # 28  f-tiles over F
S1_CHUNK = 512    # stage-1 moving-dim chunk
HEAD_W = 3        # w1/w3 f-tiles issued ahead of the loop (prefetch depth)
WARM = 24         # 256-col warm-up matmuls bridging preamble -> first data
C_CAP = 1024      # device token capacity; overflow beyond this is tiny and
                  # computed on host

_nc_cache = {}


def _chunks(C, step):
    out = []
    off = 0
    while off < C:
        sz = min(step, C - off)
        out.append((off, sz))
        off += sz
    return out


def _s1_chunks(C):
    # x is laid out chunk-major on the host, so every chunk DMA is one
    # contiguous block per partition (few descriptors): strided x transfers
    # measured ~9us issue-to-semaphore on either DGE path and stalled the
    # whole core.  The first chunk is 256 tokens so its completion semaphore
    # (the gate for the first real matmul group) fires ~1.5us sooner than a
    # 512 chunk's would.
    if C > 512:
        return [(0, 256)] + [(o + 256, s) for o, s in _chunks(C - 256, S1_CHUNK)]
    return _chunks(C, S1_CHUNK)


def _build(C):
    f16, f32 = mybir.dt.float16, mybir.dt.float32
    s1_chunks = _s1_chunks(C)
    TT = C // 128  # token tiles for stage 2

    nc = bacc.Bacc("TRN2", target_bir_lowering=False, debug=False, num_devices=E)
    xb = nc.dram_tensor("xb", [128, KO * C], f16, kind="ExternalInput")
    w1b = nc.dram_tensor("w1b", [FT, 128, KO, 128], f16, kind="ExternalInput")
    w3b = nc.dram_tensor("w3b", [FT, 128, KO, 128], f16, kind="ExternalInput")
    w2b = nc.dram_tensor("w2b", [128, FT, H], f16, kind="ExternalInput")
    yb = nc.dram_tensor("yb", [C, H], f16, kind="ExternalOutput")

    with tile.TileContext(nc) as tc:
        with (
            tc.tile_pool(name="xpool", bufs=1) as xpool,
            tc.tile_pool(name="actpool", bufs=1) as actpool,
            tc.tile_pool(name="w13pool", bufs=4) as w13pool,
            tc.tile_pool(name="w2pool", bufs=1) as w2pool,
            tc.tile_pool(name="outpool", bufs=4) as outpool,
            tc.tile_pool(name="silupool", bufs=4) as silupool,
            tc.tile_pool(name="warmpool", bufs=1) as warmpool,
            tc.tile_pool(name="ps1", bufs=3, space="PSUM") as ps1,
            tc.tile_pool(name="ps2", bufs=2, space="PSUM") as ps2,
        ):
            # Head of the sync queue: the f=0 weights the first matmul group
            # needs, then a HEAD_W-deep prefetch so the PE never starves
            # while the DMA pipeline builds steady state.
            head_w = []
            # First f-tile w1 is split into ko-halves for ring pipelining.
            # The x chunk rides ALONE on the scalar ring so its completion
            # receipt is as fast as possible; all other head traffic goes
            # on sync.  DMA completion receipts degrade when the rings are
            # saturated, so the head burst is kept small (HEAD_W pairs,
            # pool caps the rest).
            w1t0 = w13pool.tile([128, KO, 128], f16, tag="w1t", name="w1t0")
            nc.sync.dma_start(w1t0[:, : KO // 2], w1b[0][:, : KO // 2])
            xt = xpool.tile([128, KO * C], f16)
            co, cs = s1_chunks[0]
            # Chunk 0 rides ALONE on the scalar ring as ONE piece: receipts
            # land ~2.8us after the transfer regardless, and a ko-split gate
            # that lets the first group start on half a chunk was measured
            # to stall mid-group (and reset the clock ramp) whenever the
            # second half's receipt slipped.
            nc.scalar.dma_start(
                xt[:, KO * co : KO * (co + cs)], xb[:, KO * co : KO * (co + cs)]
            )
            nc.sync.dma_start(w1t0[:, KO // 2 :], w1b[0][:, KO // 2 :])
            # w3 f=0 in ko-halves too: its ko0-3 matmuls (stream+0.85us)
            # then gate on half the bytes, shrinking the late-receipt
            # stall observed at the first group's w1->w3 handoff.
            w3t0 = w13pool.tile([128, KO, 128], f16, tag="w3t", name="w3t0")
            nc.sync.dma_start(w3t0[:, : KO // 2], w3b[0][:, : KO // 2])
            nc.sync.dma_start(w3t0[:, KO // 2 :], w3b[0][:, KO // 2 :])
            head_w.append((w1t0, w3t0))
            # Prefetch pairs BEFORE the bulk x chunks: with the chunk-0-
            # first stage-1 order, f1/f2 weights are needed at stream+1.7us
            # and +3.4us while the x chunks 1/2 are not needed until
            # stream+5.1us and +8.5us — so the weight pairs must beat the
            # 1.5MB of bulk x onto the sync ring.  Each bulk chunk is
            # ko-split so its first matmul group gates on half the bytes.
            for f in range(1, HEAD_W):
                w1tf = w13pool.tile([128, KO, 128], f16, tag="w1t", name="w1th")
                nc.sync.dma_start(w1tf[:], w1b[f])
                w3tf = w13pool.tile([128, KO, 128], f16, tag="w3t", name="w3th")
                nc.sync.dma_start(w3tf[:], w3b[f])
                head_w.append((w1tf, w3tf))
            # Chunk 1 rides the scalar ring behind chunk 0 (the ring is
            # otherwise idle until the stage-2 output tiles ~200us later),
            # so its receipt beats the stream+5.1us deadline instead of
            # queueing behind ~3MB of weight traffic on sync.  Chunk 2+
            # stays on sync (deadline stream+8.5us, met comfortably).
            for ci, (co, cs) in enumerate(s1_chunks[1:], start=1):
                q = nc.scalar if ci == 1 else nc.sync
                half = KO // 2 * cs
                q.dma_start(
                    xt[:, KO * co : KO * co + half], xb[:, KO * co : KO * co + half]
                )
                q.dma_start(
                    xt[:, KO * co + half : KO * (co + cs)],
                    xb[:, KO * co + half : KO * (co + cs)],
                )

            # Warm-up matmuls bridge the ~4us between the engine-sync
            # barrier and the first weight DMA's completion semaphore, so
            # the PE HAM activity window opens as early as possible and the
            # real stream starts the moment data is visible.
            warm = warmpool.tile([128, 512], mybir.dt.bfloat16)
            nc.vector.memset(warm[:], 0.0)
            # WARM 256-col warm-up matmuls bridge preamble-end to the
            # arrival of the first x/weight completion semaphores — fewer
            # leaves the PE idle at the handoff, more overshoots past
            # data-ready.  256-col granularity keeps the overshoot small.
            for _ in range(WARM):
                wp = ps2.tile([128, 512], f32, tag="py", name="wp")
                nc.tensor.matmul(
                    wp[:, :256], warm[:, :128], warm[:, :256], start=True, stop=True
                )

            act = actpool.tile([128, FT, C], f16, tag="act")
            w2t = w2pool.tile([128, FT, H], f16, tag="w2t")

            # Stage 1: actT[f, c] = silu(w1 xT) * (w3 xT), per 128-row f tile.
            # Iteration order: chunk 0 across the HEAD_W resident f-tiles
            # FIRST (gated only on the head DMAs), then their chunks 1/2,
            # then f >= HEAD_W in normal order.  This pushes the x chunk-1
            # receipt deadline from stream+1.7us to stream+5.1us, so no
            # filler matmuls are needed and a late receipt cannot gap the
            # PE.  Within the catch-up block f-major order (f0c1, f0c2,
            # f1c1, ...) retires f0's weight tile early so f3's pool
            # buffer recycle (WAR on f0's last read) cannot delay its DMA.
            nch = len(s1_chunks)
            if nch > 1:
                s1_order = [(f, 0) for f in range(HEAD_W)]
                s1_order += [
                    (f, ci) for f in range(HEAD_W) for ci in range(1, nch)
                ]
                s1_order += [
                    (f, ci) for f in range(HEAD_W, FT) for ci in range(nch)
                ]
            else:
                s1_order = [(f, 0) for f in range(FT)]
            cur_w = {}
            for f, ci in s1_order:
                co, cs = s1_chunks[ci]
                if f < HEAD_W:
                    w1t, w3t = head_w[f]
                elif f in cur_w:
                    w1t, w3t = cur_w[f]
                else:
                    w1t = w13pool.tile([128, KO, 128], f16, tag="w1t", name="w1t")
                    nc.sync.dma_start(w1t[:], w1b[f])
                    w3t = w13pool.tile([128, KO, 128], f16, tag="w3t", name="w3t")
                    nc.sync.dma_start(w3t[:], w3b[f])
                    cur_w = {f: (w1t, w3t)}
                    # Interleave w2 in 4-f-tile batches behind the weight
                    # stream; all of w2 is resident well before stage 2.
                    fw = f - HEAD_W
                    if fw % 4 == 0 and fw + 4 <= FT:
                        nc.sync.dma_start(w2t[:, fw : fw + 4], w2b[:, fw : fw + 4])
                if True:
                    p1 = ps1.tile([128, S1_CHUNK], f32, tag="p1", name="p1")[:, :cs]
                    p3 = ps1.tile([128, S1_CHUNK], f32, tag="p3", name="p3")[:, :cs]
                    for ko in range(KO):
                        xs = xt[:, KO * co + ko * cs : KO * co + (ko + 1) * cs]
                        nc.tensor.matmul(
                            p1, w1t[:, ko], xs,
                            start=(ko == 0), stop=(ko == KO - 1),
                        )
                    for ko in range(KO):
                        xs = xt[:, KO * co + ko * cs : KO * co + (ko + 1) * cs]
                        nc.tensor.matmul(
                            p3, w3t[:, ko], xs,
                            start=(ko == 0), stop=(ko == KO - 1),
                        )
                    st = silupool.tile([128, S1_CHUNK], f32, tag="st", name="st")[:, :cs]
                    nc.scalar.activation(
                        st, p1, mybir.ActivationFunctionType.Silu
                    )
                    nc.vector.tensor_tensor(
                        act[:, f, co : co + cs], st, p3, mybir.AluOpType.mult
                    )
            # (w2 fully covered by the 4-f-tile batches above: fw = 0,4,..,24)

            # Stage 2: y[tok, h] += actT[:, tok-tile].T @ w2T[:, h-block],
            # accumulating all 28 f-tiles in one PSUM bank.
            for t in range(TT):
                ts = slice(t * 128, (t + 1) * 128)
                for hb in range(HB):
                    hs = slice(hb * 512, (hb + 1) * 512)
                    last = t == TT - 1 and hb == HB - 1
                    if not last:
                        py = ps2.tile([128, 512], f32, tag="py", name="py")
                        for kf in range(FT):
                            nc.tensor.matmul(
                                py, act[:, kf, ts], w2t[:, kf, hs],
                                start=(kf == 0), stop=(kf == FT - 1),
                            )
                        osb = outpool.tile([128, 512], f16, tag="osb", name="osb")
                        nc.vector.tensor_copy(osb[:], py[:])
                        # Alternate output rings so consecutive tiles' write
                        # receipts overlap.
                        q = nc.sync if hb == 0 else nc.scalar
                        q.dma_start(yb[ts, hs], osb[:])
                    else:
                        # Final tile 384/128-split: the 384-col cast+DMA
                        # hide behind the 128-col group's matmuls, so the
                        # post-stream tail is only a 128-col cast plus a
                        # 32KB write instead of a full 512-col tile.
                        h0 = hb * 512
                        pya = ps2.tile([128, 512], f32, tag="py", name="pya")
                        for kf in range(FT):
                            nc.tensor.matmul(
                                pya[:, :384], act[:, kf, ts], w2t[:, kf, h0 : h0 + 384],
                                start=(kf == 0), stop=(kf == FT - 1),
                            )
                        osba = outpool.tile([128, 512], f16, tag="osb", name="osba")
                        nc.vector.tensor_copy(osba[:, :384], pya[:, :384])
                        nc.scalar.dma_start(yb[ts, h0 : h0 + 384], osba[:, :384])
                        pyb = ps2.tile([128, 512], f32, tag="py", name="pyb")
                        for kf in range(FT):
                            nc.tensor.matmul(
                                pyb[:, :128], act[:, kf, ts],
                                w2t[:, kf, h0 + 384 : h0 + 512],
                                start=(kf == 0), stop=(kf == FT - 1),
                            )
                        osbb = outpool.tile([128, 512], f16, tag="osb", name="osbb")
                        nc.vector.tensor_copy(osbb[:, :128], pyb[:, :128])
                        nc.sync.dma_start(yb[ts, h0 + 384 : h0 + 512], osbb[:, :128])
    nc.compile()
    return nc


def _routing(x, gate_w):
    """Replicates the reference router in fp32 numpy: softmax over expert
    logits, top-2, renormalized weights.  Verified to match jax bit-for-bit
    on expert selection for these inputs (min top2/top3 prob gap 3e-5)."""
    logits = x @ gate_w.T
    m = logits.max(-1, keepdims=True)
    p = np.exp(logits - m)
    p /= p.sum(-1, keepdims=True)
    top_i = np.argsort(-p, axis=-1, kind="stable")[:, :TOP_K]
    top_v = np.take_along_axis(p, top_i, axis=-1)
    top_v = top_v / top_v.sum(-1, keepdims=True)
    return top_i, top_v


def kernel(hidden_states, gate_w, w1, w3, w2):
    B, S, _ = hidden_states.shape
    x = np.ascontiguousarray(
        np.asarray(hidden_states, dtype=np.float32).reshape(-1, H)
    )
    gate_w = np.asarray(gate_w, dtype=np.float32)
    w1 = np.asarray(w1, dtype=np.float32)
    w3 = np.asarray(w3, dtype=np.float32)
    w2 = np.asarray(w2, dtype=np.float32)
    T = x.shape[0]

    top_i, top_v = _routing(x, gate_w)

    idx = [np.flatnonzero((top_i == e).any(axis=1)) for e in range(E)]
    wgt = []
    for e in range(E):
        sel = top_i[idx[e]] == e
        wgt.append(
            np.take_along_axis(top_v[idx[e]], np.argmax(sel, 1)[:, None], 1)[:, 0]
        )

    cmax = max(len(i) for i in idx)
    C = min(max(((cmax + 127) // 128) * 128, 128), C_CAP)
    n_dev = [min(len(i), C) for i in idx]

    if C not in _nc_cache:
        _nc_cache[C] = _build(C)
    nc = _nc_cache[C]

    in_maps = []
    for e in range(E):
        x_pad = np.zeros((C, H), dtype=np.float32)
        x_pad[: n_dev[e]] = x[idx[e][: n_dev[e]]]
        # Chunk-major flat layout [128 partition, KO*C]: each 512-token
        # chunk is one contiguous block per partition, so its DMA needs few
        # descriptors and completes (semaphore included) promptly.
        xb = np.concatenate(
            [
                x_pad[co : co + cs]
                .T.reshape(KO, 128, cs)
                .transpose(1, 0, 2)
                .reshape(128, KO * cs)
                for co, cs in _s1_chunks(C)
            ],
            axis=1,
        )
        t1 = w1[e].reshape(FT, 128, KO, 128)
        w1b = np.ascontiguousarray(t1.transpose(0, 3, 2, 1))
        t3 = w3[e].reshape(FT, 128, KO, 128)
        w3b = np.ascontiguousarray(t3.transpose(0, 3, 2, 1))
        w2b = np.ascontiguousarray(w2[e].T.reshape(FT, 128, H).transpose(1, 0, 2))
        in_maps.append({"xb": xb.astype(np.float16), "w1b": w1b.astype(np.float16), "w3b": w3b.astype(np.float16), "w2b": w2b.astype(np.float16)})

    res = run_bass_kernel_spmd(nc, in_maps, core_ids=list(range(E)))

    out = np.zeros((T, H), dtype=np.float32)
    for e in range(E):
        y_e = res.results[e]["yb"].astype(np.float32)  # [C, H] fp16 on device
        out[idx[e][: n_dev[e]]] += wgt[e][: n_dev[e], None] * y_e[: n_dev[e]]
        if len(idx[e]) > n_dev[e]:
            # Overflow tokens past the capacity grid (a percent or so in the
            # worst-loaded expert): exact fp32 on host.
            xo = x[idx[e][n_dev[e] :]]
            h1 = xo @ w1[e].T
            a = (h1 / (1.0 + np.exp(-h1))) * (xo @ w3[e].T)
            yo = a @ w2[e].T
            out[idx[e][n_dev[e] :]] += wgt[e][n_dev[e] :, None] * yo
    return out.reshape(B, S, H)

